# revision 10
# baseline (speedup 1.0000x reference)
"""BigBird block-sparse attention for Trainium2, 8-core SPMD.

Sharding: head-parallel. Each core owns 2 of the 16 heads (both batches).
  - q/k projections computed only for the core's 128 feature slice
    (full hidden_states replicated, weights sliced column-wise).
  - v is computed directly in TRANSPOSED (token-major) layout via
    matmuls (out[tok, feat] = hT^T @ wvT per 128-token block), so the
    separate transpose phase of the v path disappears and the PE stays
    continuously busy (avoids the HAM clock down-gate that a tensor-idle
    phase triggers).
  - attention fully local per (batch, head).
  - out_proj tensor-parallel on the head (contraction) dim: each core
    emits a full-shape partial; the host sums the 8 partials and adds
    the output bias.

On-device layout choices:
  - activations feature-major (features on partitions, tokens on free dim)
  - scores computed transposed: S_T[key, query] = k_j^T q, so that
    * AV is a natural matmul (contraction = keys = partitions),
    * the softmax denominator Z falls out of a ones-column appended to V^T,
    * normalization folds into the PSUM->SBUF context copy as a
      partition-broadcast multiply by 1/Z.
  - softmax skips max-subtraction (scores are O(1) after the 1/8 scale;
    exp cannot overflow fp32 for this distribution; softmax is shift
    invariant so the reference is matched).
  - BigBird mask is data independent and block-constant (64x64): it is
    evaluated at trace time into run-lists of attending query blocks per
    128-wide key tile.  No mask tensors on device at all.
  - AV pieces: full key tiles use vaug2 (128 keys + ones col, K=128);
    even half-blocks use the TOP HALF of vaug2 (K=64, base partition 0 —
    legal); only odd half-blocks need a zero-padded slot (vodd) because
    base-partition-64 contraction operands hit a codegen/HW bug.
  - out_proj for batch 0 is interleaved into the attention phase (after
    pair (1,0)) so the finalize chain of the last pair is hidden behind
    real PE work instead of an idle tail.
"""

import numpy as np
import ml_dtypes
from contextlib import ExitStack

# ----- problem constants (hardcoded per contract) --------------------------
EMBED_DIM = 1024
NUM_HEADS = 16
HEAD_DIM = 64           # d per head
WINDOW = 3
N_RAND = 3
BLOCK = 64
BATCH = 2
SEQ = 2048
NB = SEQ // BLOCK       # 32 key/query blocks per sequence
N_CORES = 8
HPC = NUM_HEADS // N_CORES      # heads per core = 2
FPC = HPC * HEAD_DIM            # feature slice per core = 128
T = BATCH * SEQ                 # 4096 tokens
NKT = NB // 2                   # 16 key tiles of 128 keys per (b,h)
SCALE = HEAD_DIM ** -0.5

BF16 = ml_dtypes.bfloat16

# score-chunk window width in psum columns (2 PSUM banks)
CHUNK_W = 1024
PSUM_BANK = 512  # fp32 elements per bank


def _block_attend() -> np.ndarray:
    """attend[r, kb]: query block r attends key block kb.

    Block-granular replica of the reference _bigbird_mask (the mask is
    block-constant: global first block rows/cols, +-WINDOW band, and
    N_RAND random blocks per row drawn with RandomState(0))."""
    att = np.zeros((NB, NB), dtype=bool)
    att[0, :] = True
    att[:, 0] = True
    blk = np.arange(NB)
    att |= np.abs(blk[:, None] - blk[None, :]) <= WINDOW
    rng = np.random.RandomState(0)
    for b in range(1, NB):
        avail = [x for x in range(1, NB) if abs(x - b) > WINDOW]
        if avail:
            sel = rng.choice(avail, size=min(N_RAND, len(avail)), replace=False)
            att[b, sel] = True
    return att


def _runs_of(mask_1d: np.ndarray):
    """[(r0, nblocks)] maximal runs of consecutive True entries."""
    runs = []
    for r in np.flatnonzero(mask_1d):
        if runs and runs[-1][0] + runs[-1][1] == r:
            runs[-1][1] += 1
        else:
            runs.append([int(r), 1])
    return [(r0, n) for r0, n in runs]


def build_schedule():
    """Per key-tile j, pack score pieces into <=CHUNK_W-wide psum windows.

    Returns list of chunks; each chunk is a dict:
      j      : key tile index (keys j*128 .. j*128+128)
      W      : used width in psum columns
      segs   : [(side, r0, nblk, off)]  real score/AV pieces
               side: 2=full tile (128 keys), 0=low half (kb 2j), 1=high half
               r0   : first query block, nblk consecutive blocks
               off  : chunk-local psum column offset (64*... aligned)
      fillers: [(side, off, w)] dummy score MMs so exp never reads
               unwritten psum (output discarded; AV never touches them)
    """
    att = _block_attend()
    chunks = []
    for j in range(NKT):
        kb0, kb1 = 2 * j, 2 * j + 1
        a0, a1 = att[:, kb0], att[:, kb1]
        # full pieces first, then the two half-piece streams sharing columns
        full_runs = _runs_of(a0 & a1)
        h0_runs = _runs_of(a0 & ~a1)
        h1_runs = _runs_of(a1 & ~a0)

        # absolute column layout: full region, then overlap region for halves
        layout = []  # (side, r0, nblk, abs_off)
        off = 0
        for r0, n in full_runs:
            layout.append((2, r0, n, off))
            off += 64 * n
        half_base = off
        off0 = off1 = half_base
        for r0, n in h0_runs:
            layout.append((0, r0, n, off0))
            off0 += 64 * n
        for r0, n in h1_runs:
            layout.append((1, r0, n, off1))
            off1 += 64 * n
        W_total = max(off0, off1)

        # slice the absolute layout into CHUNK_W windows
        n_windows = max(1, -(-W_total // CHUNK_W))
        for w_i in range(n_windows):
            lo, hi = w_i * CHUNK_W, min((w_i + 1) * CHUNK_W, W_total)
            segs = []
            cover = [np.zeros(hi - lo, bool), np.zeros(hi - lo, bool)]
            for side, r0, n, aoff in layout:
                s, e = aoff, aoff + 64 * n
                cs, ce = max(s, lo), min(e, hi)
                if cs >= ce:
                    continue
                # clip to window; r advances with columns (64 per block)
                r_lo = r0 + (cs - s) // 64
                nblk = (ce - cs) // 64
                segs.append((side, r_lo, nblk, cs - lo))
                for sd in ((0, 1) if side == 2 else (side,)):
                    cover[sd][cs - lo:ce - lo] = True
            if not segs:
                continue
            used = max(o + 64 * n for (_s, _r, n, o) in segs)
            fillers = []
            for sd in (0, 1):
                m = ~cover[sd][:used]
                i = 0
                while i < used:
                    if m[i]:
                        k = i
                        while k < used and m[k]:
                            k += 1
                        fillers.append((sd, i, k - i))
                        i = k
                    else:
                        i += 1
            chunks.append(dict(j=j, W=used, segs=segs, fillers=fillers))
    return chunks


def _bank_split(off, w, bank=PSUM_BANK):
    """split [off, off+w) at bank boundaries -> [(off, w), ...]"""
    out = []
    while w > 0:
        room = bank - (off % bank)
        take = min(room, w)
        out.append((off, take))
        off += take
        w -= take
    return out


# ---------------------------------------------------------------------------
# numpy golden of the exact on-device algorithm (fp32, validates schedule)
# ---------------------------------------------------------------------------
def numpy_golden(hidden_states, wq, bq, wk, bk, wv, bv, wo, bo):
    hs = np.asarray(hidden_states, np.float32).reshape(T, EMBED_DIM)
    chunks = build_schedule()
    out = np.zeros((T, EMBED_DIM), np.float32)
    for c in range(N_CORES):
        f = slice(FPC * c, FPC * (c + 1))
        q = hs @ np.asarray(wq, np.float32)[f, :].T  # (T, 128)
        k = hs @ np.asarray(wk, np.float32)[f, :].T
        v = hs @ np.asarray(wv, np.float32)[f, :].T
        partial = np.zeros((EMBED_DIM, T), np.float32)
        ctx_all = np.zeros((FPC, T), np.float32)
        for b in range(BATCH):
            for hl in range(HPC):
                d = slice(64 * hl, 64 * hl + 64)
                tok = slice(b * SEQ, (b + 1) * SEQ)
                qb = q[tok, d]   # (2048, 64)
                kb = k[tok, d]
                vb = v[tok, d]
                v_aug = np.concatenate([vb, np.ones((SEQ, 1), np.float32)], 1)
                ctx = np.zeros((65, SEQ), np.float32)
                for ch in chunks:
                    j = ch["j"]
                    E = np.zeros((128, ch["W"]), np.float32)
                    for side, r0, nblk, off in ch["segs"]:
                        kk = (slice(j * 128, j * 128 + 128) if side == 2 else
                              slice(j * 128 + 64 * side, j * 128 + 64 * side + 64))
                        qq = slice(64 * r0, 64 * (r0 + nblk))
                        s = kb[kk, :] @ qb[qq, :].T  # (keys, queries)
                        E[0 if side in (0, 2) else 64:][:s.shape[0], off:off + 64 * nblk] = \
                            np.exp(SCALE * s)
                    for side, r0, nblk, off in ch["segs"]:
                        kk = (slice(j * 128, j * 128 + 128) if side == 2 else
                              slice(j * 128 + 64 * side, j * 128 + 64 * side + 64))
                        rows = slice(0, 128) if side == 2 else \
                            slice(64 * side, 64 * side + 64)
                        qq = slice(64 * r0, 64 * (r0 + nblk))
                        ctx[:, qq] += v_aug[kk, :].T @ E[rows, off:off + 64 * nblk]
                ctx_n = ctx[:64, :] / ctx[64:65, :]
                ctx_all[d, tok] = ctx_n
        partial = np.asarray(wo, np.float32)[:, f] @ ctx_all  # (1024, T)
        out += partial.T
    out = out + np.asarray(bo, np.float32)
    return out.reshape(BATCH, SEQ, EMBED_DIM)


# ---------------------------------------------------------------------------
# Bass/Tile kernel (one core's program; SPMD across 8 cores)
# ---------------------------------------------------------------------------
def _trace_core_program():
    import concourse.bass as bass
    import concourse.mybir as mybir
    import concourse.tile as tile
    from concourse import bacc

    dt = mybir.dt
    chunks = build_schedule()

    nc = bacc.Bacc(None, target_bir_lowering=False)
    with tile.TileContext(nc) as tc:
        with ExitStack() as top:
            dram = top.enter_context(tc.tile_pool(name="dram", bufs=1, space="DRAM"))
            hT_d = dram.tile([EMBED_DIM, T], dt.bfloat16, kind="ExternalInput",
                             name="hT", uniquify=False)
            wqkT_d = dram.tile([EMBED_DIM, 2 * FPC], dt.bfloat16,
                               kind="ExternalInput", name="wqkT", uniquify=False)
            wvT_d = dram.tile([EMBED_DIM, FPC], dt.bfloat16,
                              kind="ExternalInput", name="wvT", uniquify=False)
            woT_d = dram.tile([FPC, EMBED_DIM], dt.bfloat16,
                              kind="ExternalInput", name="woT", uniquify=False)
            out_d = dram.tile([EMBED_DIM, T], dt.bfloat16,
                              kind="ExternalOutput", name="out", uniquify=False)

            # ---- persistent SBUF tensors -----------------------------------
            persist = top.enter_context(tc.tile_pool(name="persist", bufs=1))
            wqk = persist.tile([128, 8, 2 * FPC], dt.bfloat16, name="wqk_sb")
            wvT = persist.tile([128, 8, FPC], dt.bfloat16, name="wv_sb")
            woT = persist.tile([128, EMBED_DIM], dt.bfloat16, name="wo_sb")
            # q/k head-major on 64 partitions (base-0 only: matmuls with
            # base-partition-64 contraction operands hit a codegen/HW bug)
            q_sb = persist.tile([64, HPC * T], dt.bfloat16, name="q_sb")
            k_sb = persist.tile([64, HPC * T], dt.bfloat16, name="k_sb")
            # per (b,hl): full-tile [v|1] slots (both blocks interleaved on
            # 128 partitions), used K=128 for full pieces and K=64 (top half
            # = even block) for even half pieces
            vaug2 = persist.tile([128, BATCH * HPC, NKT * 65], dt.bfloat16,
                                 name="vaug2_sb")
            # odd half pieces: rows 64:128 hold [v_odd | 1], rows 0:64 zero
            # (so K=128 at base partition 0 works and the garbage top rows
            # of E multiply by zero)
            vodd = persist.tile([128, BATCH * HPC, NKT * 65], dt.bfloat16,
                                name="vodd_sb")
            ctx_all = persist.tile([128, T], dt.bfloat16, name="ctx_sb")
            zt = persist.tile([128, 256], dt.bfloat16, name="zt_sb")

            # weight DMAs first (gpsimd queue, needed ~2-4us in)
            nc.gpsimd.dma_start(out=wqk[:], in_=wqkT_d.rearrange(
                "(e p) f -> p e f", p=128))
            nc.gpsimd.dma_start(out=wvT[:], in_=wvT_d.rearrange(
                "(e p) f -> p e f", p=128))
            nc.gpsimd.dma_start(out=woT[:], in_=woT_d[:])

            NCHUNK = T // 512
            hT_pool = tc.tile_pool(name="hT_pool", bufs=1)
            with hT_pool as hp:
                hT = hp.tile([128, 8, T], dt.bfloat16, name="hT_sb")

                # chunk-0 hT DMAs first so projection can start ASAP;
                # dispatches spread over 4 engine queues (descriptor gen is
                # ~600ns of engine time apiece — one queue would serialize)
                def h_dma(n, e):
                    eng = (nc.sync if e < 4 else
                           nc.scalar if e < 6 else nc.gpsimd)
                    eng.dma_start(
                        out=hT[:, e, 512 * n:512 * n + 512],
                        in_=hT_d[128 * e:128 * e + 128, 512 * n:512 * n + 512])

                for e in range(8):
                    h_dma(0, e)

                # ~3.5us of dummy matmuls while DMAs stream: flips the HAM
                # clock gate to 8/8 so the projection starts at full clock.
                # Also writes every cell of all 8 PSUM banks once (launders
                # boot-garbage psum).  Closed before proj pools open so the
                # 8 banks are free again.
                nc.vector.memset(zt[:], 0.0)
                with tc.tile_pool(name="warm_ps", bufs=8, space="PSUM") as wps:
                    for i in range(8):
                        w = wps.tile([128, 512], dt.float32, tag="warm")
                        for hf in range(2):
                            nc.tensor.matmul(w[:, 256 * hf:256 * hf + 256],
                                             zt[:, 0:128], zt[:, 0:256],
                                             start=True, stop=True,
                                             skip_group_check=True)

                # presets for vaug2/vodd (cheap; contiguous/strided memsets)
                nc.gpsimd.memset(vodd[0:64, :, :], 0.0)
                for p in range(BATCH * HPC):
                    s2 = vaug2[:, p, :].rearrange("p (m c) -> p m c", c=65)
                    nc.vector.memset(s2[:, :, 64:65], 1.0)
                    so = vodd[:, p, :].rearrange("p (m c) -> p m c", c=65)
                    nc.gpsimd.memset(so[64:128, :, 64:65], 1.0)

                # remaining hT DMAs in (chunk, e) order
                for n in range(1, NCHUNK):
                    for e in range(8):
                        h_dma(n, e)

                # ---- phase 1: q/k projections + direct v^T -----------------
                # psum->sbuf copies: only DVE and ACT can read PSUM
                cp_rot = [nc.vector.tensor_copy, nc.scalar.copy]
                cp_i = [0]

                def cp(dst, src):
                    cp_rot[cp_i[0] % 2](dst, src)
                    cp_i[0] += 1

                with tc.tile_pool(name="proj_ps", bufs=2, space="PSUM") as pps, \
                        tc.tile_pool(name="vt_ps", bufs=4, space="PSUM") as vtp:
                    for n in range(NCHUNK):
                        tsl = slice(512 * n, 512 * (n + 1))
                        # q and k (feature-major, head-split into q_sb/k_sb)
                        for wsl, dst in [(slice(0, 128), q_sb),
                                         (slice(128, 256), k_sb)]:
                            ps = pps.tile([128, 512], dt.float32, tag="proj")
                            for e in range(8):
                                nc.tensor.matmul(ps[:], wqk[:, e, wsl],
                                                 hT[:, e, tsl],
                                                 start=(e == 0), stop=(e == 7))
                            for hl in range(HPC):
                                cp(dst[:, hl * T + 512 * n:
                                       hl * T + 512 * n + 512],
                                   ps[64 * hl:64 * hl + 64, :])
                        # v^T for the 4 token blocks of this chunk: psum
                        # [128 tok, 128 feat(h0|h1)], then strided copies
                        # into the vaug2/vodd slots of both heads at once
                        for gg in range(4):
                            g = 4 * n + gg
                            b, jj = g // NKT, g % NKT
                            vt = vtp.tile([128, 128], dt.float32, tag="vt")
                            for e in range(8):
                                nc.tensor.matmul(
                                    vt[:], hT[:, e, 128 * g:128 * g + 128],
                                    wvT[:, e, :], start=(e == 0),
                                    stop=(e == 7))
                            p0 = b * HPC
                            cp(vaug2[:, p0:p0 + 2, 65 * jj:65 * jj + 64],
                               vt[:].rearrange("p (h c) -> p h c", h=2))
                            cp(vodd[64:128, p0:p0 + 2, 65 * jj:65 * jj + 64],
                               vt[64:128, :].rearrange("p (h c) -> p h c",
                                                       h=2))

            # ---- attention + interleaved out-projection --------------------
            def emit_pair(b, hl, scp, ctxp, ep, fp, tag):
                p = b * HPC + hl
                qtok0 = hl * T + b * SEQ  # column base in q/k (head-major)
                ctok0 = b * SEQ           # column base in ctx_all
                ctx = ctxp.tile([65, SEQ], dt.float32, tag="ctx")
                # PSUM start=True arms the whole bank for lazy zeroing:
                # issue it exactly once per ctx bank (j=0 covers every
                # column, so all banks start during the j=0 chunks).
                ctx_bank_started = [False] * (SEQ // PSUM_BANK)

                # ALL scores+exp first, then ALL AV: the AV block of this
                # pair waits on the previous pair's finalize (ctx psum
                # reuse), and PE is in-order — front-loading the score
                # matmuls hides that chain.
                E_tiles = []
                for ci, ch in enumerate(chunks):
                    j, W = ch["j"], ch["W"]
                    S = scp.tile([128, CHUNK_W], dt.float32, tag="S")
                    E = ep.tile([128, W], dt.bfloat16,
                                tag=f"E{tag}{ci}", name=f"E{tag}{ci}")
                    E_tiles.append(E)
                    kcol0 = qtok0 + 128 * j

                    def k_lhsT(side):
                        if side == 2:
                            return k_sb[:, kcol0:kcol0 + 128]
                        return k_sb[:, kcol0 + 64 * side:
                                    kcol0 + 64 * side + 64]

                    def s_rows(side):
                        return (slice(0, 128) if side == 2
                                else slice(64 * side, 64 * side + 64))

                    # scores (+fillers), split at psum banks; h0/h1
                    # alternated so their disjoint PE column groups can
                    # overlap in the array
                    sfull = [g for g in ch["segs"] if g[0] == 2]
                    sh = [g for g in ch["segs"] if g[0] == 0]
                    sh1 = [g for g in ch["segs"] if g[0] == 1]
                    inter = []
                    for a_, b_ in zip(sh, sh1):
                        inter += [a_, b_]
                    longer = sh if len(sh) > len(sh1) else sh1
                    inter += longer[min(len(sh), len(sh1)):]
                    for side, r0, nblk, off in sfull + inter:
                        for o, w in _bank_split(off, 64 * nblk):
                            qc = qtok0 + 64 * r0 + (o - off)
                            nc.tensor.matmul(
                                S[s_rows(side), o:o + w],
                                k_lhsT(side),
                                q_sb[:, qc:qc + w],
                                start=True, stop=True)
                    for side, off, w in ch["fillers"]:
                        for o, ww in _bank_split(off, w):
                            nc.tensor.matmul(
                                S[s_rows(side), o:o + ww],
                                k_lhsT(side),
                                q_sb[:, qtok0:qtok0 + ww],
                                start=True, stop=True)
                    # exp
                    nc.scalar.activation(
                        E[:, :W], S[:, :W],
                        mybir.ActivationFunctionType.Exp, scale=SCALE)
                # AV accumulate (+Z via ones column)
                for ci, ch in enumerate(chunks):
                    j = ch["j"]
                    E = E_tiles[ci]
                    for side, r0, nblk, off in ch["segs"]:
                        if side == 2:
                            lhsT = vaug2[:, p, 65 * j:65 * j + 65]
                            erows = slice(0, 128)
                        elif side == 0:
                            lhsT = vaug2[0:64, p, 65 * j:65 * j + 65]
                            erows = slice(0, 64)
                        else:
                            lhsT = vodd[:, p, 65 * j:65 * j + 65]
                            erows = slice(0, 128)
                        for o, w in _bank_split(64 * r0, 64 * nblk):
                            eo = off + (o - 64 * r0)
                            bank = o // PSUM_BANK
                            st = not ctx_bank_started[bank]
                            ctx_bank_started[bank] = True
                            nc.tensor.matmul(
                                ctx[:, o:o + w], lhsT,
                                E[erows, eo:eo + w],
                                start=st, stop=False,
                                skip_group_check=True)
                # finalize: 1/Z (spread over 64 partitions via DMA reshape:
                # a (1,2048) reciprocal is single-lane and slow), then
                # broadcast-multiply into ctx_all
                zrow = fp.tile([1, SEQ], dt.float32, tag="zrow")
                nc.vector.tensor_copy(zrow[:], ctx[64:65, :])
                zsp = fp.tile([64, SEQ // 64], dt.float32, tag="zsp")
                nc.sync.dma_start(out=zsp[:], in_=zrow[:])
                rsp = fp.tile([64, SEQ // 64], dt.float32, tag="rsp")
                nc.vector.reciprocal(rsp[:], zsp[:])
                rrow = fp.tile([1, SEQ], dt.float32, tag="rrow")
                nc.sync.dma_start(out=rrow[:], in_=rsp[:])
                rbc = fp.tile([64, SEQ], dt.float32, tag="rbc")
                nc.gpsimd.partition_broadcast(rbc[:], rrow[:])
                for cc in range(SEQ // 512):
                    csl = slice(512 * cc, 512 * (cc + 1))
                    nc.vector.tensor_tensor(
                        out=ctx_all[64 * hl:64 * hl + 64,
                                    ctok0 + 512 * cc:
                                    ctok0 + 512 * cc + 512],
                        in0=ctx[0:64, csl],
                        in1=rbc[:, csl],
                        op=mybir.AluOpType.mult)

            def emit_oproj(opp, opsb, eo_list, b, engines):
                # 2-bank psum tiles: 2 matmuls share one wide copy (fewer
                # psum->sbuf instructions; only DVE/ACT can read PSUM)
                ei = 0
                for eo in eo_list:
                    ob = opsb.tile([128, SEQ], dt.bfloat16, tag="ob")
                    for nn in range(SEQ // 1024):
                        ps = opp.tile([128, 1024], dt.float32, tag="op")
                        for half in range(2):
                            csl = slice(b * SEQ + 1024 * nn + 512 * half,
                                        b * SEQ + 1024 * nn + 512 * half + 512)
                            nc.tensor.matmul(
                                ps[:, 512 * half:512 * half + 512],
                                woT[:, 128 * eo:128 * eo + 128],
                                ctx_all[:, csl], start=True, stop=True)
                        engines[ei % len(engines)](
                            ob[:, 1024 * nn:1024 * nn + 1024], ps[:])
                        ei += 1
                    nc.sync.dma_start(
                        out=out_d[128 * eo:128 * eo + 128,
                                  b * SEQ:(b + 1) * SEQ],
                        in_=ob[:])

            # segment 1: pairs (0,0),(0,1),(1,0) — scp before ctxp so the
            # out-proj pool that follows lands on the score banks (whose
            # last readers finished long ago), not the ctx banks
            with tc.tile_pool(name="sc_ps", bufs=2, space="PSUM") as scp, \
                    tc.tile_pool(name="ctx_ps", bufs=1, space="PSUM") as ctxp, \
                    tc.tile_pool(name="e_pool", bufs=1) as ep, \
                    tc.tile_pool(name="fin_pool", bufs=2) as fp:
                emit_pair(0, 0, scp, ctxp, ep, fp, "a")
                emit_pair(0, 1, scp, ctxp, ep, fp, "a")
                emit_pair(1, 0, scp, ctxp, ep, fp, "a")

            # out-proj batch 0, first half of eo (runs while pair (1,0)'s
            # finalize chain completes; depends only on batch-0 ctx)
            with tc.tile_pool(name="op_ps1", bufs=3, space="PSUM") as opp, \
                    tc.tile_pool(name="op_sb1", bufs=2) as opsb:
                emit_oproj(opp, opsb, range(0, 4), 0,
                           [nc.vector.tensor_copy, nc.scalar.copy])

            # segment 2: last pair (1,1)
            with tc.tile_pool(name="sc_ps2", bufs=2, space="PSUM") as scp, \
                    tc.tile_pool(name="ctx_ps2", bufs=1, space="PSUM") as ctxp, \
                    tc.tile_pool(name="e_pool2", bufs=1) as ep, \
                    tc.tile_pool(name="fin_pool2", bufs=1) as fp:
                emit_pair(1, 1, scp, ctxp, ep, fp, "b")

            # out-proj: rest of batch 0 (covers the last finalize chain),
            # then batch 1
            with tc.tile_pool(name="op_ps2", bufs=3, space="PSUM") as opp, \
                    tc.tile_pool(name="op_sb2", bufs=3) as opsb:
                emit_oproj(opp, opsb, range(4, 8), 0,
                           [nc.vector.tensor_copy, nc.scalar.copy])
                emit_oproj(opp, opsb, range(0, 8), 1,
                           [nc.vector.tensor_copy, nc.scalar.copy])

    nc.compile()
    return nc


_NC_CACHE = None


def make_in_maps(hs, wq, wk, wv, wo):
    hT = np.ascontiguousarray(
        np.asarray(hs, np.float32).reshape(T, EMBED_DIM).T).astype(BF16)
    wq = np.asarray(wq, np.float32)
    wk = np.asarray(wk, np.float32)
    wv = np.asarray(wv, np.float32)
    wo = np.asarray(wo, np.float32)
    in_maps = []
    for c in range(N_CORES):
        f = slice(FPC * c, FPC * (c + 1))
        wqkT = np.concatenate([wq[f, :].T, wk[f, :].T], axis=1)  # (1024, 256)
        in_maps.append({
            "hT": hT,
            "wqkT": np.ascontiguousarray(wqkT).astype(BF16),
            "wvT": np.ascontiguousarray(wv[f, :].T).astype(BF16),
            "woT": np.ascontiguousarray(wo[:, f].T).astype(BF16),
        })
    return in_maps


def kernel(hidden_states, wq, bq, wk, bk, wv, bv, wo, bo):
    global _NC_CACHE
    hs = np.asarray(hidden_states, np.float32)
    wq = np.asarray(wq, np.float32)
    wk = np.asarray(wk, np.float32)
    wv = np.asarray(wv, np.float32)
    wo = np.asarray(wo, np.float32)
    bq = np.asarray(bq, np.float32)
    bk = np.asarray(bk, np.float32)
    bv = np.asarray(bv, np.float32)
    bo = np.asarray(bo, np.float32)
    assert hs.shape == (BATCH, SEQ, EMBED_DIM)
    # biases bq/bk/bv are zero in this problem; fold nonzero ones on host
    # by shifting is impossible (they pass through nonlinearities), so
    # guard loudly rather than silently returning wrong results.
    for name, bias in (("bq", bq), ("bk", bk), ("bv", bv)):
        if np.abs(bias).max() != 0:
            raise NotImplementedError(f"nonzero {name} not supported")

    from concourse.bass_utils import run_bass_kernel_spmd

    if _NC_CACHE is None:
        _NC_CACHE = _trace_core_program()
    nc = _NC_CACHE

    in_maps = make_in_maps(hs, wq, wk, wv, wo)
    res = run_bass_kernel_spmd(nc, in_maps, list(range(N_CORES)))
    acc = np.zeros((EMBED_DIM, T), np.float32)
    for c in range(N_CORES):
        acc += res.results[c]["out"].astype(np.float32)
    out = acc.T + bo[None, :]
    return out.reshape(BATCH, SEQ, EMBED_DIM).astype(np.float32)


# revision 13
# speedup vs baseline: 1.0408x; 1.0408x over previous
"""BigBird block-sparse attention for Trainium2, 8-core SPMD.

Sharding: head-parallel. Each core owns 2 of the 16 heads (both batches).
  - q/k/v projections computed only for the core's 128 feature slice
    (full hidden_states replicated, weights sliced column-wise).
  - attention fully local per (batch, head).
  - out_proj tensor-parallel on the head (contraction) dim: each core
    emits a full-shape partial; the host sums the 8 partials and adds
    the output bias.

Performance model notes (measured on HW): the PE is INSTRUCTION-ISSUE
bound at ~110ns per matmul when matmuls are narrow — wide matmuls are
essential.  The attention schedule therefore uses UNION runs: for each
128-key tile, one full-height score matmul per run of consecutive
query blocks attending EITHER half (64-key block) of the tile.  Query
blocks attending only one half get the other half's exp values zeroed
in SBUF (cheap DVE/Pool memsets) so that a single K=128 AV matmul per
run is correct — the appended ones column then also yields the correct
softmax denominator Z.

On-device layout choices:
  - activations feature-major (features on partitions, tokens on free dim)
  - scores computed transposed: S_T[key, query] = k_j^T q, so that
    * AV is a natural matmul (contraction = keys = partitions),
    * the softmax denominator Z falls out of a ones-column appended to V^T,
    * normalization folds into the PSUM->SBUF context copy as a
      partition-broadcast multiply by 1/Z.
  - softmax skips max-subtraction (scores are O(1) after the 1/8 scale;
    exp cannot overflow fp32 for this distribution; softmax is shift
    invariant so the reference is matched).
  - v is projected feature-major on all 128 partitions (both heads) and
    transposed per 128-token tile with ONE PE transpose, then copied to
    the [v | 1] slots (vaug2) both heads at once.
  - out_proj for batch 0 is interleaved into the attention phase so the
    finalize chain of the last pair is hidden behind real PE work.
"""

import numpy as np
import ml_dtypes
from contextlib import ExitStack

# ----- problem constants (hardcoded per contract) --------------------------
EMBED_DIM = 1024
NUM_HEADS = 16
HEAD_DIM = 64           # d per head
WINDOW = 3
N_RAND = 3
BLOCK = 64
BATCH = 2
SEQ = 2048
NB = SEQ // BLOCK       # 32 key/query blocks per sequence
N_CORES = 8
HPC = NUM_HEADS // N_CORES      # heads per core = 2
FPC = HPC * HEAD_DIM            # feature slice per core = 128
T = BATCH * SEQ                 # 4096 tokens
NKT = NB // 2                   # 16 key tiles of 128 keys per (b,h)
SCALE = HEAD_DIM ** -0.5

BF16 = ml_dtypes.bfloat16

# score-chunk window width in psum columns (2 PSUM banks)
CHUNK_W = 1024
PSUM_BANK = 512  # fp32 elements per bank


def _block_attend() -> np.ndarray:
    """attend[r, kb]: query block r attends key block kb.

    Block-granular replica of the reference _bigbird_mask (the mask is
    block-constant: global first block rows/cols, +-WINDOW band, and
    N_RAND random blocks per row drawn with RandomState(0))."""
    att = np.zeros((NB, NB), dtype=bool)
    att[0, :] = True
    att[:, 0] = True
    blk = np.arange(NB)
    att |= np.abs(blk[:, None] - blk[None, :]) <= WINDOW
    rng = np.random.RandomState(0)
    for b in range(1, NB):
        avail = [x for x in range(1, NB) if abs(x - b) > WINDOW]
        if avail:
            sel = rng.choice(avail, size=min(N_RAND, len(avail)), replace=False)
            att[b, sel] = True
    return att


def _runs_of(mask_1d: np.ndarray):
    """[(r0, nblocks)] maximal runs of consecutive True entries."""
    runs = []
    for r in np.flatnonzero(mask_1d):
        if runs and runs[-1][0] + runs[-1][1] == r:
            runs[-1][1] += 1
        else:
            runs.append([int(r), 1])
    return [(r0, n) for r0, n in runs]


def build_schedule():
    """Union-run schedule, packed into CHUNK_W-wide psum windows.

    Returns list of chunks; each chunk is a dict:
      W     : used width in psum columns
      runs  : [(j, r0, nblk, off)] score/AV runs — query blocks
              r0..r0+nblk attend at least one half of key tile j; the
              score matmul is full-height (128 keys), AV is one K=128
              matmul per run against the [v|1] slot of tile j.
      zeros : [(row0, nrows, off, w)] exp outputs to zero in E (the
              unattended half of single-side columns).
    """
    att = _block_attend()
    chunks = []
    cur = dict(W=0, runs=[], zeros=[])
    for j in range(NKT):
        a0, a1 = att[:, 2 * j], att[:, 2 * j + 1]
        u = a0 | a1
        for r0, n in _runs_of(u):
            while n > 0:
                space = (CHUNK_W - cur["W"]) // 64
                if space == 0:
                    chunks.append(cur)
                    cur = dict(W=0, runs=[], zeros=[])
                    continue
                take = min(n, space)
                off = cur["W"]
                cur["runs"].append((j, r0, take, off))
                # zero rects for single-side query blocks, merged along
                # consecutive same-type blocks
                qq = r0
                while qq < r0 + take:
                    t = (2 if (a0[qq] and a1[qq]) else (0 if a0[qq] else 1))
                    q2 = qq
                    while q2 < r0 + take and \
                            (2 if (a0[q2] and a1[q2]) else
                             (0 if a0[q2] else 1)) == t:
                        q2 += 1
                    if t != 2:
                        # even-only (t=0): zero odd rows 64:128;
                        # odd-only (t=1): zero even rows 0:64
                        cur["zeros"].append((64 * (1 - t), 64,
                                             off + 64 * (qq - r0),
                                             64 * (q2 - qq)))
                    qq = q2
                cur["W"] += 64 * take
                r0 += take
                n -= take
    if cur["runs"]:
        chunks.append(cur)
    return chunks


def _bank_split(off, w, bank=PSUM_BANK):
    """split [off, off+w) at bank boundaries -> [(off, w), ...]"""
    out = []
    while w > 0:
        room = bank - (off % bank)
        take = min(room, w)
        out.append((off, take))
        off += take
        w -= take
    return out


# ---------------------------------------------------------------------------
# numpy golden of the exact on-device algorithm (fp32, validates schedule)
# ---------------------------------------------------------------------------
def numpy_golden(hidden_states, wq, bq, wk, bk, wv, bv, wo, bo):
    hs = np.asarray(hidden_states, np.float32).reshape(T, EMBED_DIM)
    chunks = build_schedule()
    out = np.zeros((T, EMBED_DIM), np.float32)
    for c in range(N_CORES):
        f = slice(FPC * c, FPC * (c + 1))
        q = hs @ np.asarray(wq, np.float32)[f, :].T  # (T, 128)
        k = hs @ np.asarray(wk, np.float32)[f, :].T
        v = hs @ np.asarray(wv, np.float32)[f, :].T
        ctx_all = np.zeros((FPC, T), np.float32)
        for b in range(BATCH):
            for hl in range(HPC):
                d = slice(64 * hl, 64 * hl + 64)
                tok = slice(b * SEQ, (b + 1) * SEQ)
                qb = q[tok, d]   # (2048, 64)
                kb = k[tok, d]
                vb = v[tok, d]
                v_aug = np.concatenate([vb, np.ones((SEQ, 1), np.float32)], 1)
                ctx = np.zeros((65, SEQ), np.float32)
                for ch in chunks:
                    E = np.zeros((128, ch["W"]), np.float32)
                    for j, r0, nblk, off in ch["runs"]:
                        kk = slice(j * 128, j * 128 + 128)
                        qq = slice(64 * r0, 64 * (r0 + nblk))
                        s = kb[kk, :] @ qb[qq, :].T  # (128 keys, queries)
                        E[:, off:off + 64 * nblk] = np.exp(SCALE * s)
                    for row0, nrows, off, w in ch["zeros"]:
                        E[row0:row0 + nrows, off:off + w] = 0.0
                    for j, r0, nblk, off in ch["runs"]:
                        kk = slice(j * 128, j * 128 + 128)
                        qq = slice(64 * r0, 64 * (r0 + nblk))
                        ctx[:, qq] += v_aug[kk, :].T @ E[:, off:off + 64 * nblk]
                ctx_n = ctx[:64, :] / ctx[64:65, :]
                ctx_all[d, tok] = ctx_n
        partial = np.asarray(wo, np.float32)[:, f] @ ctx_all  # (1024, T)
        out += partial.T
    out = out + np.asarray(bo, np.float32)
    return out.reshape(BATCH, SEQ, EMBED_DIM)


# ---------------------------------------------------------------------------
# Bass/Tile kernel (one core's program; SPMD across 8 cores)
# ---------------------------------------------------------------------------
def _trace_core_program():
    import concourse.bass as bass
    import concourse.mybir as mybir
    import concourse.tile as tile
    from concourse import bacc

    dt = mybir.dt
    chunks = build_schedule()

    nc = bacc.Bacc(None, target_bir_lowering=False)
    with tile.TileContext(nc) as tc:
        with ExitStack() as top:
            dram = top.enter_context(tc.tile_pool(name="dram", bufs=1, space="DRAM"))
            hT_d = dram.tile([EMBED_DIM, T], dt.bfloat16, kind="ExternalInput",
                             name="hT", uniquify=False)
            wqkvT_d = dram.tile([EMBED_DIM, 3 * FPC], dt.bfloat16,
                                kind="ExternalInput", name="wqkvT",
                                uniquify=False)
            woT_d = dram.tile([FPC, EMBED_DIM], dt.bfloat16,
                              kind="ExternalInput", name="woT", uniquify=False)
            ident_d = dram.tile([128, 128], dt.bfloat16,
                                kind="ExternalInput", name="ident",
                                uniquify=False)
            out_d = dram.tile([EMBED_DIM, T], dt.bfloat16,
                              kind="ExternalOutput", name="out", uniquify=False)

            # ---- persistent SBUF tensors -----------------------------------
            persist = top.enter_context(tc.tile_pool(name="persist", bufs=1))
            wqkv = persist.tile([128, 8, 3 * FPC], dt.bfloat16, name="wqkv_sb")
            woT = persist.tile([128, EMBED_DIM], dt.bfloat16, name="wo_sb")
            ident = persist.tile([128, 128], dt.bfloat16, name="ident_sb")
            # q/k head-major on 64 partitions (base-0 only: matmuls with
            # base-partition-64 contraction operands hit a codegen/HW bug)
            q_sb = persist.tile([64, HPC * T], dt.bfloat16, name="q_sb")
            k_sb = persist.tile([64, HPC * T], dt.bfloat16, name="k_sb")
            # v feature-major, both heads on 128 partitions (transposed
            # per-tile on the PE)
            vfm = persist.tile([128, T], dt.bfloat16, name="vfm_sb")
            # per (b,hl): [v | 1] slots per 128-key tile, K=128 for all AV
            vaug2 = persist.tile([128, BATCH * HPC, NKT * 65], dt.bfloat16,
                                 name="vaug2_sb")
            ctx_all = persist.tile([128, T], dt.bfloat16, name="ctx_sb")
            zt = persist.tile([128, 256], dt.bfloat16, name="zt_sb")

            # weight DMAs first (gpsimd queue, needed ~2-4us in)
            nc.gpsimd.dma_start(out=wqkv[:], in_=wqkvT_d.rearrange(
                "(e p) f -> p e f", p=128))
            nc.gpsimd.dma_start(out=woT[:], in_=woT_d[:])
            nc.gpsimd.dma_start(out=ident[:], in_=ident_d[:])

            NCHUNK = T // 512
            hT_pool = tc.tile_pool(name="hT_pool", bufs=1)
            with hT_pool as hp:
                hT = hp.tile([128, 8, T], dt.bfloat16, name="hT_sb")

                # hT DMA dispatches spread over 3 engine queues (descriptor
                # generation is ~600ns of engine time apiece); all issued
                # before any memset/copy work so the queues drain into DMA
                # as early as possible
                def h_dma(n, e):
                    eng = (nc.sync if e < 4 else
                           nc.scalar if e < 6 else nc.gpsimd)
                    eng.dma_start(
                        out=hT[:, e, 512 * n:512 * n + 512],
                        in_=hT_d[128 * e:128 * e + 128, 512 * n:512 * n + 512])

                for e in range(8):
                    h_dma(0, e)

                # ~3us of dummy matmuls while DMAs stream: keeps the PE
                # pipeline warm so the projection starts at full p-state,
                # and writes every cell of all 8 PSUM banks once (launders
                # boot-garbage psum).  Closed before proj pools open.
                nc.vector.memset(zt[:], 0.0)
                with tc.tile_pool(name="warm_ps", bufs=8, space="PSUM") as wps:
                    for i in range(8):
                        w = wps.tile([128, 512], dt.float32, tag="warm")
                        for hf in range(2):
                            nc.tensor.matmul(w[:, 256 * hf:256 * hf + 256],
                                             zt[:, 0:128], zt[:, 0:256],
                                             start=True, stop=True,
                                             skip_group_check=True)

                for n in range(1, NCHUNK):
                    for e in range(8):
                        h_dma(n, e)

                # vaug2 ones-columns preset (cheap strided memsets on DVE)
                for p in range(BATCH * HPC):
                    s2 = vaug2[:, p, :].rearrange("p (m c) -> p m c", c=65)
                    nc.vector.memset(s2[:, :, 64:65], 1.0)

                # ---- phase 1: q/k/v projections + per-tile v transpose -----
                # psum->sbuf copies: only DVE and ACT can read PSUM
                cp_rot = [nc.vector.tensor_copy, nc.scalar.copy]
                cp_i = [0]

                def cp(dst, src):
                    cp_rot[cp_i[0] % 2](dst, src)
                    cp_i[0] += 1

                with tc.tile_pool(name="proj_ps", bufs=2, space="PSUM") as pps, \
                        tc.tile_pool(name="vt_ps", bufs=2, space="PSUM") as vtp:
                    for n in range(NCHUNK):
                        tsl = slice(512 * n, 512 * (n + 1))
                        # q and k (feature-major, head-split into q_sb/k_sb)
                        for tg, wsl, dst in [("pq", slice(0, 128), q_sb),
                                             ("pk", slice(128, 256), k_sb)]:
                            ps = pps.tile([128, 512], dt.float32, tag=tg)
                            for e in range(8):
                                nc.tensor.matmul(ps[:], wqkv[:, e, wsl],
                                                 hT[:, e, tsl],
                                                 start=(e == 0), stop=(e == 7))
                            for hl in range(HPC):
                                cp(dst[:, hl * T + 512 * n:
                                       hl * T + 512 * n + 512],
                                   ps[64 * hl:64 * hl + 64, :])
                        # v: both heads on 128 partitions, single wide copy
                        ps = pps.tile([128, 512], dt.float32, tag="pv")
                        for e in range(8):
                            nc.tensor.matmul(ps[:], wqkv[:, e, 256:384],
                                             hT[:, e, tsl],
                                             start=(e == 0), stop=(e == 7))
                        cp(vfm[:, tsl], ps[:])
                        # transpose the 4 completed 128-token tiles of the
                        # PREVIOUS chunk (vfm write must land first; lag one
                        # chunk so the copy has completed)
                        for gg in range(4):
                            g = 4 * (n - 1) + gg if n > 0 else None
                            if g is None:
                                break
                            b, jj = g // NKT, g % NKT
                            vt = vtp.tile([128, 128], dt.bfloat16, tag="vt")
                            nc.tensor.transpose(
                                vt[:], vfm[:, 128 * g:128 * g + 128],
                                ident[:])
                            p0 = b * HPC
                            cp(vaug2[:, p0:p0 + 2, 65 * jj:65 * jj + 64],
                               vt[:].rearrange("p (h c) -> p h c", h=2))
                    # last chunk's 4 tiles
                    for gg in range(4):
                        g = 4 * (NCHUNK - 1) + gg
                        b, jj = g // NKT, g % NKT
                        vt = vtp.tile([128, 128], dt.bfloat16, tag="vt")
                        nc.tensor.transpose(
                            vt[:], vfm[:, 128 * g:128 * g + 128], ident[:])
                        p0 = b * HPC
                        cp(vaug2[:, p0:p0 + 2, 65 * jj:65 * jj + 64],
                           vt[:].rearrange("p (h c) -> p h c", h=2))

            # ---- attention + interleaved out-projection --------------------
            def emit_pair(b, hl, scp, ctxp, ep, fp, tag):
                p = b * HPC + hl
                qtok0 = hl * T + b * SEQ  # column base in q/k (head-major)
                ctok0 = b * SEQ           # column base in ctx_all
                ctx = ctxp.tile([65, SEQ], dt.float32, tag="ctx")
                # PSUM start=True arms the whole bank for lazy zeroing:
                # issue it exactly once per ctx bank (the j=0 runs cover
                # every query column, so all banks start early).
                ctx_bank_started = [False] * (SEQ // PSUM_BANK)

                # ALL scores+exp+zeroing first, then ALL AV: the AV block
                # of this pair waits on the previous pair's finalize (ctx
                # psum reuse), and PE is in-order — front-loading the score
                # matmuls hides that chain.
                E_tiles = []
                zi = 0
                for ci, ch in enumerate(chunks):
                    W = ch["W"]
                    S = scp.tile([128, CHUNK_W], dt.float32, tag="S")
                    E = ep.tile([128, W], dt.bfloat16,
                                tag=f"E{tag}{ci}", name=f"E{tag}{ci}")
                    E_tiles.append(E)
                    for j, r0, nblk, off in ch["runs"]:
                        kcol0 = qtok0 + 128 * j
                        for o, w in _bank_split(off, 64 * nblk):
                            qc = qtok0 + 64 * r0 + (o - off)
                            nc.tensor.matmul(
                                S[:, o:o + w],
                                k_sb[:, kcol0:kcol0 + 128],
                                q_sb[:, qc:qc + w],
                                start=True, stop=True)
                    nc.scalar.activation(
                        E[:, :W], S[:, :W],
                        mybir.ActivationFunctionType.Exp, scale=SCALE)
                    # zero the unattended half of single-side columns
                    for row0, nrows, off, w in ch["zeros"]:
                        (nc.vector if zi % 2 else nc.gpsimd).memset(
                            E[row0:row0 + nrows, off:off + w], 0.0)
                        zi += 1
                # AV accumulate (+Z via ones column), K=128 always
                for ci, ch in enumerate(chunks):
                    E = E_tiles[ci]
                    for j, r0, nblk, off in ch["runs"]:
                        lhsT = vaug2[:, p, 65 * j:65 * j + 65]
                        for o, w in _bank_split(64 * r0, 64 * nblk):
                            eo = off + (o - 64 * r0)
                            bank = o // PSUM_BANK
                            st = not ctx_bank_started[bank]
                            ctx_bank_started[bank] = True
                            nc.tensor.matmul(
                                ctx[:, o:o + w], lhsT,
                                E[:, eo:eo + w],
                                start=st, stop=False,
                                skip_group_check=True)
                # finalize: 1/Z (spread over 64 partitions via DMA reshape:
                # a (1,2048) reciprocal is single-lane and slow), then
                # broadcast-multiply into ctx_all.  zrow copy on ACT so the
                # DVE queue isn't blocked ahead of out-proj copies.
                zrow = fp.tile([1, SEQ], dt.float32, tag="zrow")
                nc.scalar.copy(zrow[:], ctx[64:65, :])
                zsp = fp.tile([64, SEQ // 64], dt.float32, tag="zsp")
                nc.sync.dma_start(out=zsp[:], in_=zrow[:])
                rsp = fp.tile([64, SEQ // 64], dt.float32, tag="rsp")
                nc.vector.reciprocal(rsp[:], zsp[:])
                rrow = fp.tile([1, SEQ], dt.float32, tag="rrow")
                nc.sync.dma_start(out=rrow[:], in_=rsp[:])
                rbc = fp.tile([64, SEQ], dt.float32, tag="rbc")
                nc.gpsimd.partition_broadcast(rbc[:], rrow[:])
                for cc in range(SEQ // 512):
                    csl = slice(512 * cc, 512 * (cc + 1))
                    nc.vector.tensor_tensor(
                        out=ctx_all[64 * hl:64 * hl + 64,
                                    ctok0 + 512 * cc:
                                    ctok0 + 512 * cc + 512],
                        in0=ctx[0:64, csl],
                        in1=rbc[:, csl],
                        op=mybir.AluOpType.mult)

            def emit_oproj(opp, opsb, eo_list, b, engines):
                # 2-bank psum tiles: 2 matmuls share one wide copy (fewer
                # psum->sbuf instructions; only DVE/ACT can read PSUM)
                ei = 0
                for eo in eo_list:
                    ob = opsb.tile([128, SEQ], dt.bfloat16, tag="ob")
                    for nn in range(SEQ // 1024):
                        ps = opp.tile([128, 1024], dt.float32, tag="op")
                        for half in range(2):
                            csl = slice(b * SEQ + 1024 * nn + 512 * half,
                                        b * SEQ + 1024 * nn + 512 * half + 512)
                            nc.tensor.matmul(
                                ps[:, 512 * half:512 * half + 512],
                                woT[:, 128 * eo:128 * eo + 128],
                                ctx_all[:, csl], start=True, stop=True)
                        engines[ei % len(engines)](
                            ob[:, 1024 * nn:1024 * nn + 1024], ps[:])
                        ei += 1
                    nc.sync.dma_start(
                        out=out_d[128 * eo:128 * eo + 128,
                                  b * SEQ:(b + 1) * SEQ],
                        in_=ob[:])

            # segment 1: pairs (0,0),(0,1),(1,0) — scp before ctxp so the
            # out-proj pool that follows lands on the score banks (whose
            # last readers finished long ago), not the ctx banks
            with tc.tile_pool(name="sc_ps", bufs=2, space="PSUM") as scp, \
                    tc.tile_pool(name="ctx_ps", bufs=1, space="PSUM") as ctxp, \
                    tc.tile_pool(name="e_pool", bufs=1) as ep, \
                    tc.tile_pool(name="fin_pool", bufs=2) as fp:
                emit_pair(0, 0, scp, ctxp, ep, fp, "a")
                emit_pair(0, 1, scp, ctxp, ep, fp, "a")
                emit_pair(1, 0, scp, ctxp, ep, fp, "a")

            # out-proj batch 0, first half of eo (runs while pair (1,0)'s
            # finalize chain completes; copies on ACT so the DVE-resident
            # finalize chain doesn't block them)
            with tc.tile_pool(name="op_ps1", bufs=3, space="PSUM") as opp, \
                    tc.tile_pool(name="op_sb1", bufs=2) as opsb:
                emit_oproj(opp, opsb, range(0, 4), 0, [nc.scalar.copy])

            # segment 2: last pair (1,1)
            with tc.tile_pool(name="sc_ps2", bufs=2, space="PSUM") as scp, \
                    tc.tile_pool(name="ctx_ps2", bufs=1, space="PSUM") as ctxp, \
                    tc.tile_pool(name="e_pool2", bufs=1) as ep, \
                    tc.tile_pool(name="fin_pool2", bufs=1) as fp:
                emit_pair(1, 1, scp, ctxp, ep, fp, "b")

            # out-proj: rest of batch 0 (covers the last finalize chain,
            # ACT copies), then batch 1 (alternating engines)
            with tc.tile_pool(name="op_ps2", bufs=3, space="PSUM") as opp, \
                    tc.tile_pool(name="op_sb2", bufs=3) as opsb:
                emit_oproj(opp, opsb, range(4, 8), 0, [nc.scalar.copy])
                emit_oproj(opp, opsb, range(0, 8), 1,
                           [nc.scalar.copy, nc.vector.tensor_copy])

    nc.compile()
    return nc


_NC_CACHE = None


def make_in_maps(hs, wq, wk, wv, wo):
    hT = np.ascontiguousarray(
        np.asarray(hs, np.float32).reshape(T, EMBED_DIM).T).astype(BF16)
    ident = np.eye(128, dtype=np.float32).astype(BF16)
    wq = np.asarray(wq, np.float32)
    wk = np.asarray(wk, np.float32)
    wv = np.asarray(wv, np.float32)
    wo = np.asarray(wo, np.float32)
    in_maps = []
    for c in range(N_CORES):
        f = slice(FPC * c, FPC * (c + 1))
        wqkvT = np.concatenate([wq[f, :].T, wk[f, :].T, wv[f, :].T], axis=1)
        in_maps.append({
            "hT": hT,
            "wqkvT": np.ascontiguousarray(wqkvT).astype(BF16),
            "woT": np.ascontiguousarray(wo[:, f].T).astype(BF16),
            "ident": ident,
        })
    return in_maps


def kernel(hidden_states, wq, bq, wk, bk, wv, bv, wo, bo):
    global _NC_CACHE
    hs = np.asarray(hidden_states, np.float32)
    wq = np.asarray(wq, np.float32)
    wk = np.asarray(wk, np.float32)
    wv = np.asarray(wv, np.float32)
    wo = np.asarray(wo, np.float32)
    bq = np.asarray(bq, np.float32)
    bk = np.asarray(bk, np.float32)
    bv = np.asarray(bv, np.float32)
    bo = np.asarray(bo, np.float32)
    assert hs.shape == (BATCH, SEQ, EMBED_DIM)
    # biases bq/bk/bv are zero in this problem; fold nonzero ones on host
    # by shifting is impossible (they pass through nonlinearities), so
    # guard loudly rather than silently returning wrong results.
    for name, bias in (("bq", bq), ("bk", bk), ("bv", bv)):
        if np.abs(bias).max() != 0:
            raise NotImplementedError(f"nonzero {name} not supported")

    from concourse.bass_utils import run_bass_kernel_spmd

    if _NC_CACHE is None:
        _NC_CACHE = _trace_core_program()
    nc = _NC_CACHE

    in_maps = make_in_maps(hs, wq, wk, wv, wo)
    res = run_bass_kernel_spmd(nc, in_maps, list(range(N_CORES)))
    acc = np.zeros((EMBED_DIM, T), np.float32)
    for c in range(N_CORES):
        acc += res.results[c]["out"].astype(np.float32)
    out = acc.T + bo[None, :]
    return out.reshape(BATCH, SEQ, EMBED_DIM).astype(np.float32)
